# revision 3
# baseline (speedup 1.0000x reference)
"""MultiHeadAttention + LoRA (Q,V) Trainium2 kernel.

Sharding: data-parallel over batch. 16 batches / 8 cores = 2 batches/core.
Weights replicated per core; no collectives.

Per-core layout: activations are feature-major [feature, token] (1024 tokens =
2 batches x 512). Host pre-transposes x and all weight matrices (fp32 DMA
transpose is unsupported on-device). Softmax: scores are computed transposed
(S^T [tk, tq]), exp on ACT, key-mask folded into V (masked V rows zeroed), and
an extra mask-column appended to V makes the P@V matmul emit the softmax
denominator as one extra PSUM row. Matmuls run in float32r (full-rate fp32,
~1e-4 rounding).
"""

import sys

sys.path.insert(0, "/opt/trn_rl_repo")

import numpy as np

import concourse.bass as bass  # noqa: F401
import concourse.mybir as mybir
import concourse.tile as tile
from concourse import bacc
from concourse.bass_utils import run_bass_kernel_spmd

F32 = mybir.dt.float32
F32R = mybir.dt.float32r
AF = mybir.ActivationFunctionType
ALU = mybir.AluOpType

NC = 8          # cores
B, T, H = 16, 512, 768
NH, HD, R = 12, 64, 4
SCALING = 2.0   # lora_alpha / r
BL = B // NC    # batches per core (2)
TL = BL * T     # tokens per core (1024)
IC = H // 128   # input-feature chunks (6)
TTILES = TL // 128  # token tiles per core (8)

_CACHE = {}


def _build():
    nc = bacc.Bacc(
        "TRN2",
        target_bir_lowering=False,
        debug=False,
        enable_asserts=False,
        num_devices=NC,
    )

    def din(name, shape, dt):
        return nc.dram_tensor(name, shape, dt, kind="ExternalInput").ap()

    d = {
        "xT": din("xT", [H, TL], F32R),
        "WqT": din("WqT", [H, H], F32R),
        "WkT": din("WkT", [H, H], F32R),
        "WvT": din("WvT", [H, H], F32R),
        "WoT": din("WoT", [H, H], F32R),
        "AqT": din("AqT", [H, R], F32R),
        "AvT": din("AvT", [H, R], F32R),
        "BqT": din("BqT", [R, H], F32R),
        "BvT": din("BvT", [R, H], F32R),
        "bq": din("bq", [128, IC], F32),
        "bk": din("bk", [128, IC], F32),
        "bo": din("bo", [128, IC], F32),
        "bv_rep": din("bv_rep", [128, H], F32),
        "mask01": din("mask01", [128, TTILES], F32),
    }
    dout = nc.dram_tensor("outT", [H, TL], F32, kind="ExternalOutput").ap()

    with tile.TileContext(nc) as tc:
        _body(tc, d, dout)
    nc.compile()
    return nc


def _body(tc, d, dout):
    from contextlib import ExitStack

    nc = tc.nc
    with ExitStack() as ctx:
        cpool = ctx.enter_context(tc.tile_pool(name="const", bufs=1))
        wpool = ctx.enter_context(tc.tile_pool(name="w", bufs=2))
        qkvpool = ctx.enter_context(tc.tile_pool(name="qkv", bufs=1))
        psA = ctx.enter_context(tc.tile_pool(name="psA", bufs=2, space="PSUM"))
        stps = ctx.enter_context(tc.tile_pool(name="stps", bufs=1, space="PSUM"))
        pvps = ctx.enter_context(tc.tile_pool(name="pvps", bufs=2, space="PSUM"))
        outpool = ctx.enter_context(tc.tile_pool(name="outp", bufs=3))

        # ---- constants ----
        aqT = cpool.tile([128, IC, R], F32R, tag="aqT")
        avT = cpool.tile([128, IC, R], F32R, tag="avT")
        nc.sync.dma_start(aqT[:], d["AqT"].rearrange("(c p) r -> p c r", p=128))
        nc.sync.dma_start(avT[:], d["AvT"].rearrange("(c p) r -> p c r", p=128))
        bqT = cpool.tile([R, H], F32R, tag="bqT")
        bvT = cpool.tile([R, H], F32R, tag="bvT")
        nc.sync.dma_start(bqT[:], d["BqT"][:])
        nc.sync.dma_start(bvT[:], d["BvT"][:])
        bq = cpool.tile([128, IC], F32, tag="bq")
        bk = cpool.tile([128, IC], F32, tag="bk")
        bo = cpool.tile([128, IC], F32, tag="bo")
        nc.sync.dma_start(bq[:], d["bq"][:])
        nc.sync.dma_start(bk[:], d["bk"][:])
        nc.sync.dma_start(bo[:], d["bo"][:])
        bvr = cpool.tile([128, H], F32, tag="bvr")
        nc.sync.dma_start(bvr[:], d["bv_rep"][:])
        m01 = cpool.tile([128, TTILES], F32, tag="m01")
        nc.sync.dma_start(m01[:], d["mask01"][:])

        # ---- persistent activations ----
        QT = qkvpool.tile([128, IC, TL], F32R, tag="QT")
        KT = qkvpool.tile([128, IC, TL], F32R, tag="KT")
        # V augmented: per head 65 cols. even head: [V(64), mask] ;
        # odd head: [mask, V(64)]
        V_aug = qkvpool.tile([128, TTILES, NH, HD + 1], F32R, tag="V")
        OT_all = qkvpool.tile([128, IC, TL], F32R, tag="OT")

        with ExitStack() as xctx:
            xpool = xctx.enter_context(tc.tile_pool(name="x", bufs=1))
            xT = xpool.tile([128, IC, TL], F32R, tag="xT")
            nc.sync.dma_start(xT[:], d["xT"].rearrange("(c p) t -> p c t", p=128))
            xaq = xpool.tile([R, TL], F32R, tag="xaq")
            xav = xpool.tile([R, TL], F32R, tag="xav")

            # ---- LoRA stage 1: xa = A @ xT  -> [R, TL] ----
            for half in range(2):
                ts = slice(half * 512, (half + 1) * 512)
                for dst, aT in ((xaq, aqT), (xav, avT)):
                    ps = psA.tile([R, 512], F32, tag="p")
                    for ic in range(IC):
                        nc.tensor.matmul(
                            ps[:], aT[:, ic, :], xT[:, ic, ts],
                            start=(ic == 0), stop=(ic == IC - 1),
                        )
                    nc.vector.tensor_copy(dst[:, ts], ps[:])

            # ---- Q / K projections (feature-major) ----
            for name, W, dstT, bias, lora in (
                ("WqT", None, QT, bq, (bqT, xaq)),
                ("WkT", None, KT, bk, None),
            ):
                w = wpool.tile([128, IC, H], F32R, tag="w")
                nc.sync.dma_start(
                    w[:], d[name].rearrange("(c p) o -> p c o", p=128)
                )
                for oc in range(IC):
                    os_ = slice(oc * 128, (oc + 1) * 128)
                    for half in range(2):
                        ts = slice(half * 512, (half + 1) * 512)
                        ps = psA.tile([128, 512], F32, tag="p")
                        for ic in range(IC):
                            nc.tensor.matmul(
                                ps[:], w[:, ic, os_], xT[:, ic, ts],
                                start=(ic == 0),
                                stop=(ic == IC - 1 and lora is None),
                            )
                        if lora is not None:
                            bT, xa = lora
                            nc.tensor.matmul(
                                ps[:], bT[:, os_], xa[:, ts],
                                start=False, stop=True,
                            )
                        nc.vector.tensor_scalar_add(
                            dstT[:, oc, ts], ps[:], bias[:, oc : oc + 1]
                        )

            # ---- V projection (token-major, masked, augmented) ----
            wv = wpool.tile([128, IC, H], F32R, tag="w")
            nc.sync.dma_start(wv[:], d["WvT"].rearrange("(c p) o -> p c o", p=128))
            for tt in range(TTILES):
                tsl = slice(tt * 128, (tt + 1) * 128)
                for o0, ow in ((0, 512), (512, 256)):
                    nhh = ow // HD  # heads in this slice
                    ps = psA.tile([128, 512], F32, tag="p")
                    for ic in range(IC):
                        nc.tensor.matmul(
                            ps[:, :ow], xT[:, ic, tsl], wv[:, ic, o0 : o0 + ow],
                            start=(ic == 0), stop=False,
                        )
                    nc.tensor.matmul(
                        ps[:, :ow], xav[:, tsl], bvT[:, o0 : o0 + ow],
                        start=False, stop=True,
                    )
                    pv_v = ps[:, :ow].rearrange("p (h d) -> p h d", d=HD)
                    bv_v = bvr[:, o0 : o0 + ow].rearrange("p (h d) -> p h d", d=HD)
                    va_v = V_aug[:, tt, o0 // HD : o0 // HD + nhh, 0:HD]
                    mcol = m01[:, tt : tt + 1]
                    nc.vector.tensor_tensor(va_v, pv_v, bv_v, ALU.add)
                    nc.vector.tensor_scalar_mul(va_v, va_v, mcol)
                # mask column at col 64 for every head
                nc.vector.tensor_copy(
                    V_aug[:, tt, :, HD : HD + 1],
                    m01[:, tt : tt + 1, None].to_broadcast((128, NH, 1)),
                )

        # ---- attention (per batch, head) ----
        with ExitStack() as actx:
            ptpool = actx.enter_context(tc.tile_pool(name="pt", bufs=2))
            npool = actx.enter_context(tc.tile_pool(name="norm", bufs=2))
            for b in range(BL):
                bs = slice(b * 512, (b + 1) * 512)
                for h in range(NH):
                    par, c = h % 2, h // 2
                    pb = par * 64
                    st = stps.tile([128, 4, 512], F32, tag="st")
                    for k in range(4):
                        ks = slice(b * 512 + k * 128, b * 512 + (k + 1) * 128)
                        nc.tensor.matmul(
                            st[:, k, :], KT[pb : pb + 64, c, ks],
                            QT[pb : pb + 64, c, bs], start=True, stop=True,
                        )
                    pt = ptpool.tile([128, 4, 512], F32R, tag="pt")
                    nc.scalar.activation(pt[:], st[:], AF.Exp, scale=0.125)

                    ov = pvps.tile([128, 512], F32, tag="pv")
                    for k in range(4):
                        nc.tensor.matmul(
                            ov[0:65, :], V_aug[:, b * 4 + k, h, :], pt[:, k, :],
                            start=(k == 0), stop=(k == 3),
                        )
                    rdt = npool.tile([65, 512], F32, tag="rd")
                    nc.vector.reciprocal(rdt[64:65, :], ov[64:65, :])
                    rd0 = npool.tile([1, 512], F32, tag="rd0")
                    nc.sync.dma_start(rd0[0:1, :], rdt[64:65, :])
                    Rt = npool.tile([64, 512], F32, tag="R")
                    nc.gpsimd.partition_broadcast(Rt[:], rd0[0:1, :])
                    # odd heads: quadrant-shifted DVE write (read 0-63 -> 64-127)
                    dst = (
                        OT_all[0:64, c, bs] if par == 0 else OT_all[64:128, c, bs]
                    )
                    nc.vector.tensor_mul(dst, ov[0:64, :], Rt[0:64, :])

        # ---- output projection ----
        wo = wpool.tile([128, IC, H], F32R, tag="w")
        nc.sync.dma_start(wo[:], d["WoT"].rearrange("(c p) o -> p c o", p=128))
        doutr = dout.rearrange("(c p) t -> p c t", p=128)
        for ec in range(IC):
            es = slice(ec * 128, (ec + 1) * 128)
            for half in range(2):
                ts = slice(half * 512, (half + 1) * 512)
                ps = psA.tile([128, 512], F32, tag="p")
                for jc in range(IC):
                    nc.tensor.matmul(
                        ps[:], wo[:, jc, es], OT_all[:, jc, ts],
                        start=(jc == 0), stop=(jc == IC - 1),
                    )
                ot = outpool.tile([128, 512], F32, tag="o")
                nc.vector.tensor_scalar_add(ot[:], ps[:], bo[:, ec : ec + 1])
                nc.sync.dma_start(doutr[:, ec, ts], ot[:])


def _prep_host(x, mask, Wq, bq, Aq, Bq, Wk, bk, Wv, bv, Av, Bv, Wo, bo):
    """Build shared + per-core input maps (all float32, C-contiguous)."""
    f = np.float32
    shared = {
        "WqT": np.ascontiguousarray(Wq.T, dtype=f),
        "WkT": np.ascontiguousarray(Wk.T, dtype=f),
        "WvT": np.ascontiguousarray(Wv.T, dtype=f),
        "WoT": np.ascontiguousarray(Wo.T, dtype=f),
        "AqT": np.ascontiguousarray(Aq.T, dtype=f),
        "AvT": np.ascontiguousarray(Av.T, dtype=f),
        "BqT": np.ascontiguousarray((SCALING * Bq).T, dtype=f),
        "BvT": np.ascontiguousarray((SCALING * Bv).T, dtype=f),
        "bq": np.ascontiguousarray(np.asarray(bq, f).reshape(IC, 128).T),
        "bk": np.ascontiguousarray(np.asarray(bk, f).reshape(IC, 128).T),
        "bo": np.ascontiguousarray(np.asarray(bo, f).reshape(IC, 128).T),
        "bv_rep": np.ascontiguousarray(
            np.broadcast_to(np.asarray(bv, f), (128, H))
        ),
    }
    in_maps = []
    for c in range(NC):
        xc = np.asarray(x[c * BL : (c + 1) * BL], f).reshape(TL, H)
        mc = np.asarray(mask[c * BL : (c + 1) * BL], f).reshape(TL)
        m = dict(shared)
        m["xT"] = np.ascontiguousarray(xc.T)
        # mask01[p, tt] = mask for token tt*128+p
        m["mask01"] = np.ascontiguousarray(mc.reshape(TTILES, 128).T)
        in_maps.append(m)
    return in_maps


def kernel(**inputs):
    if "nc" not in _CACHE:
        _CACHE["nc"] = _build()
    nc = _CACHE["nc"]
    in_maps = _prep_host(**inputs)
    res = run_bass_kernel_spmd(nc, in_maps, core_ids=list(range(NC)))
    out = np.empty((B, T, H), np.float32)
    for c in range(NC):
        outT = res.results[c]["outT"]  # [H, TL]
        out[c * BL : (c + 1) * BL] = outT.T.reshape(BL, T, H)
    return out
